# revision 1
# baseline (speedup 1.0000x reference)
"""Two-layer GAT (PyG GATConv-equivalent) on 8 Trainium2 NeuronCores.

Strategy (graph/data parallel, per the sharding hint):
  - Nodes are sharded by destination across the 8 cores (N/8 each).
  - Layer-1 projection x@W0 is computed replicated (x is replicated and the
    matmul is cheap), producing on every core a full node "table" in DRAM
    with rows [h(256) | al_src(4) | al_dst(4) | pad->384] so a single
    dma_gather per edge chunk fetches everything attention needs.
  - Edge phase: per 128-dst block, edges are gathered in 128-edge chunks;
    exp(leakyrelu(al_src+al_dst)) is computed on-chip (no segment-max
    subtraction: logits are tiny for this weight scale, exp cannot
    overflow and softmax is shift-invariant), messages are ex-scaled and
    segment-summed into PSUM via a 0/1 selector-matrix matmul; the softmax
    denominator rides along as 4 extra columns; a reciprocal-multiply
    normalizes at block end.
  - Layer-2 projection runs on each core's own shard only; one 8-core
    AllGather replicates the layer-2 table; the edge phase repeats;
    mean-pool partials are AllReduced; classifier + softmax run replicated.

Host-side work is limited to graph topology preprocessing (self-loops, dst
sharding, degree-balanced 128-dst blocks, int16 gather-index slabs) and
weight folding (concatenating W@a_src / W@a_dst columns onto W); all
O(N*D) model compute runs on device.
"""
import sys

for _p in ("/opt/trn_rl_repo", "/root/.axon_site/_ro/trn_rl_repo"):
    if _p not in sys.path:
        sys.path.append(_p)

import numpy as np
import ml_dtypes

import concourse.bass as bass
import concourse.tile as tile
from concourse import bacc, mybir
from concourse.bass_utils import run_bass_kernel_spmd

F32 = mybir.dt.float32
BF16 = mybir.dt.bfloat16
I16 = mybir.dt.int16

NEG_SLOPE = 0.2
DEN_EPS = 1e-30


class Geo:
    def __init__(self, N, F, H, C, NCLS, ncores=8):
        self.N, self.F, self.H, self.C, self.NCLS = N, F, H, C, NCLS
        self.D = H * C
        self.ncores = ncores
        assert N % ncores == 0
        self.NPC = N // ncores               # nodes per core
        self.NBLK = (self.NPC + 127) // 128  # dst blocks per core
        self.LASTB = self.NPC - 128 * (self.NBLK - 1)  # rows in last block
        self.LOROWS = (N // 2 + 127) // 128 * 128      # lo/hi table split row
        assert self.LOROWS < 32768 and self.N - self.LOROWS <= 32768
        self.ROW = self.D + 2 * H            # useful row cols [h|alsrc|aldst]
        self.RPAD = ((self.ROW * 2 + 255) // 256 * 256) // 2  # padded bf16 cols
        assert (self.RPAD * 2) % 256 == 0


def block_sizes(g):
    return [128] * (g.NBLK - 1) + [g.LASTB]


# ----------------------------------------------------------------------------
# host preprocessing (topology only)
# ----------------------------------------------------------------------------

def preprocess(edge_index, g):
    import heapq
    N, NPC, NBLK = g.N, g.NPC, g.NBLK
    src = np.concatenate([edge_index[0], np.arange(N, dtype=np.int64)])
    dst = np.concatenate([edge_index[1], np.arange(N, dtype=np.int64)])
    deg = np.bincount(dst, minlength=N)

    sizes = np.array(block_sizes(g))
    blk_of = np.empty(N, np.int32)
    pos_of = np.empty(N, np.int32)
    for k in range(g.ncores):
        nodes = np.arange(k * NPC, (k + 1) * NPC)
        order = nodes[np.argsort(-deg[nodes], kind="stable")]
        cnts = np.zeros(NBLK, np.int32)
        heap = [(0.0, b) for b in range(NBLK)]
        heapq.heapify(heap)
        for n in order:
            while True:
                s, b = heapq.heappop(heap)
                if cnts[b] < sizes[b]:
                    break
            blk_of[n] = b
            pos_of[n] = cnts[b]
            cnts[b] += 1
            if cnts[b] < sizes[b]:
                heapq.heappush(heap, (s + deg[n], b))
        assert (cnts == sizes).all()

    core = np.arange(N, dtype=np.int64) // NPC
    pi = core * NPC + blk_of.astype(np.int64) * 128 + pos_of
    inv_pi = np.empty(N, np.int64)
    inv_pi[pi] = np.arange(N)

    srcrow = pi[src]
    dcore = dst // NPC
    dblk = blk_of[dst].astype(np.int64)
    dpos = pos_of[dst]
    ishi = (srcrow >= g.LOROWS).astype(np.int64)

    key = (dcore * NBLK + dblk) * 2 + ishi
    order = np.argsort(key, kind="stable")
    skey = key[order]
    ssrc = srcrow[order]
    sdpos = dpos[order]
    nkey = g.ncores * NBLK * 2
    starts = np.searchsorted(skey, np.arange(nkey))
    ends = np.searchsorted(skey, np.arange(nkey) + 1)
    cnt = (ends - starts).reshape(g.ncores, NBLK, 2)

    K_lo = [max(1, int(np.ceil(cnt[:, b, 0].max() / 128))) for b in range(NBLK)]
    K_hi = [max(1, int(np.ceil(cnt[:, b, 1].max() / 128))) for b in range(NBLK)]

    metas = []
    for k in range(g.ncores):
        idx_lo, idx_hi, dcol, drow = [], [], [], []
        for b in range(NBLK):
            for hi, Kb in ((0, K_lo[b]), (1, K_hi[b])):
                i0 = starts[(k * NBLK + b) * 2 + hi]
                i1 = ends[(k * NBLK + b) * 2 + hi]
                nn = Kb * 128
                rows = np.zeros(nn, np.int64)
                dl = np.full(nn, -1.0, np.float32)
                rows[: i1 - i0] = ssrc[i0:i1] - (g.LOROWS if hi else 0)
                dl[: i1 - i0] = sdpos[i0:i1]
                slab = np.tile(rows.reshape(-1, 16).T.astype(np.int16), (8, 1))
                (idx_hi if hi else idx_lo).append(slab)
                dcol.append(dl.reshape(Kb, 128).T)
                drow.append(dl)
        metas.append(dict(
            idx_lo=np.ascontiguousarray(np.concatenate(idx_lo, axis=1)),
            idx_hi=np.ascontiguousarray(np.concatenate(idx_hi, axis=1)),
            dstcol=np.ascontiguousarray(np.concatenate(dcol, axis=1), dtype=np.float32),
        ))
    return pi, inv_pi, K_lo, K_hi, metas


# ----------------------------------------------------------------------------
# device program
# ----------------------------------------------------------------------------

DEBUG = False


def build_program(g, K_lo, K_hi):
    NBLK, RPAD, D, H = g.NBLK, g.RPAD, g.D, g.H
    ROW = g.ROW
    NT16_lo = sum(K_lo) * 8
    NT16_hi = sum(K_hi) * 8
    NCH = sum(K_lo) + sum(K_hi)
    KMAXL, KMAXH = max(K_lo), max(K_hi)
    KMX = KMAXL + KMAXH
    ntile = (g.N + 127) // 128
    sizes = block_sizes(g)
    KC = D // 128   # feature 128-chunks (2)

    nc = bacc.Bacc(None, target_bir_lowering=False)
    dp = lambda n, s, d: nc.declare_dram_parameter(n, s, d, isOutput=False)
    xT = dp("xT", [g.F, g.N], BF16)
    W0p = dp("W0p", [g.F, ROW], BF16)
    W1p = dp("W1p", [128, KC, ROW], BF16)
    b0r = dp("b0r", [128, D], F32)
    b1r = dp("b1r", [128, D], F32)
    clsW = dp("clsW", [128, KC, g.NCLS], F32)
    clsb = dp("clsb", [1, g.NCLS], F32)
    idx_lo = dp("idx_lo", [128, NT16_lo], I16)
    idx_hi = dp("idx_hi", [128, NT16_hi], I16)
    dstcol = dp("dstcol", [128, NCH], F32)
    # consts: [iota_col | identity(128) | ones | ones_partial | iota_row(row0)]
    consts = dp("consts", [128, 259], F32)
    out_ext = nc.declare_dram_parameter("out", [1, g.NCLS], F32, isOutput=True)
    if DEBUG:
        dbg_t1 = nc.declare_dram_parameter("dbg_t1", [g.N, RPAD], BF16, isOutput=True)
        dbg_l2 = nc.declare_dram_parameter("dbg_l2", [g.NPC, RPAD], BF16, isOutput=True)
        dbg_t2 = nc.declare_dram_parameter("dbg_t2", [g.N, RPAD], BF16, isOutput=True)
        dbg_pool = nc.declare_dram_parameter("dbg_pool", [128, KC], F32, isOutput=True)
        dbg_ald = nc.declare_dram_parameter("dbg_ald", [128, NBLK, H], F32, isOutput=True)
        dbg_h1 = nc.declare_dram_parameter("dbg_h1", [g.NPC, D], F32, isOutput=True)
        dbg_S = nc.declare_dram_parameter("dbg_S", [128, NCH, 128], F32, isOutput=True)
        dbg_ST = nc.declare_dram_parameter("dbg_ST", [128, NCH, 128], F32, isOutput=True)
        dbg_rhs = nc.declare_dram_parameter("dbg_rhs", [128, NCH, D + H], F32, isOutput=True)
        dbg_alps = nc.declare_dram_parameter("dbg_alps", [128, NCH, H], F32, isOutput=True)
        dbg_g = nc.declare_dram_parameter("dbg_g", [128, NCH, RPAD], F32, isOutput=True)
        dbg_dcol = nc.declare_dram_parameter("dbg_dcol", [128, NCH], F32, isOutput=True)
        dbg_cst = nc.declare_dram_parameter("dbg_cst", [128, 259], F32, isOutput=True)
        dbg_h2 = nc.declare_dram_parameter("dbg_h2", [g.NPC, D], F32, isOutput=True)

    table1 = nc.dram_tensor("table1", [g.N, RPAD], BF16)
    l2loc = nc.dram_tensor("l2loc", [g.NPC, RPAD], BF16)
    table2 = nc.dram_tensor("table2", [g.N, RPAD], BF16, addr_space="Shared")
    ar_in = nc.dram_tensor("ar_in", [128, KC], F32)
    ar_out = nc.dram_tensor("ar_out", [128, KC], F32, addr_space="Shared")

    with tile.TileContext(nc) as tc:
      with tc.tile_pool(name="res", bufs=1) as res:
        cst = res.tile([128, 259], F32)
        nc.scalar.dma_start(out=cst[:], in_=consts[:])
        iota_col = cst[:, 0:1]
        ident = cst[:, 1:129]
        ones_f = cst[:, 129:130]
        ones_p = cst[:, 130:131]
        iota_rep = cst[:, 131:259]
        dcol_sb = res.tile([128, NCH], F32)
        nc.scalar.dma_start(out=dcol_sb[:], in_=dstcol[:])
        b0_sb = res.tile([128, D], F32)
        nc.scalar.dma_start(out=b0_sb[:], in_=b0r[:])
        b1_sb = res.tile([128, D], F32)
        nc.scalar.dma_start(out=b1_sb[:], in_=b1r[:])
        w1_sb = res.tile([128, KC, ROW], BF16)
        nc.scalar.dma_start(out=w1_sb[:], in_=W1p[:])
        w0_sb = res.tile([g.F, ROW], BF16)
        nc.scalar.dma_start(out=w0_sb[:], in_=W0p[:])
        clsw_sb = res.tile([128, KC, g.NCLS], F32)
        nc.scalar.dma_start(out=clsw_sb[:], in_=clsW[:])
        clsb_sb = res.tile([1, g.NCLS], F32)
        nc.scalar.dma_start(out=clsb_sb[:], in_=clsb[:])
        il_all = res.tile([128, NT16_lo], I16)
        nc.scalar.dma_start(out=il_all[:], in_=idx_lo[:])
        ih_all = res.tile([128, NT16_hi], I16)
        nc.scalar.dma_start(out=ih_all[:], in_=idx_hi[:])
        identb = res.tile([128, 128], BF16)
        nc.vector.tensor_copy(out=identb[:], in_=ident[:])
        stash = res.tile([128, KC, NBLK, 128], BF16)   # h1^T for L2 projection
        ald1 = res.tile([128, NBLK, H], BF16)
        ald2 = res.tile([128, NBLK, H], BF16)
        pool_sb = res.tile([128, KC], F32)

        # ---------------- phase 1: L1 projection (replicated, full N) -------
        ctx_pj = nc.named_scope("l1proj"); ctx_pj.__enter__()
        with tc.tile_pool(name="pj", bufs=3) as pj, \
             tc.tile_pool(name="pjp", bufs=2, space="PSUM") as pjp:
            GRP = 4
            t = 0
            while t < ntile:
                gn = min(GRP, ntile - t)
                r0 = t * 128
                rows = min(g.N, r0 + gn * 128) - r0
                full = [max(0, min(128, rows - i * 128)) for i in range(gn)]
                xt_sb = pj.tile([g.F, GRP, 128], BF16, tag="xt")
                nc.scalar.dma_start(
                    out=xt_sb[:, :gn, :].rearrange("p g n -> p (g n)")[:, :rows],
                    in_=xT[:, r0:r0 + rows])
                st_sb = pj.tile([128, GRP, ROW], BF16, tag="st")
                for i in range(gn):
                    if full[i] == 0:
                        continue
                    ps = pjp.tile([128, ROW], F32, tag="pp")
                    nc.tensor.matmul(out=ps[:full[i], :], lhsT=xt_sb[:, i, :full[i]],
                                     rhs=w0_sb[:], start=True, stop=True)
                    nc.vector.tensor_copy(out=st_sb[:full[i], i, :], in_=ps[:full[i], :])
                if rows == gn * 128:
                    nc.sync.dma_start(
                        out=table1[r0:r0 + rows, :ROW].rearrange("(i p) c -> p i c", p=128),
                        in_=st_sb[:, :gn, :])
                else:
                    for i in range(gn):
                        if full[i]:
                            nc.sync.dma_start(
                                out=table1[r0 + i * 128: r0 + i * 128 + full[i], :ROW],
                                in_=st_sb[:full[i], i, :])
                t += gn

        ctx_pj.__exit__(None, None, None)
        # aldst slab for layer 1: own-shard rows of the replicated table1
        pid = nc.partition_id()
        own0 = pid * g.NPC
        nfull = 128 * (NBLK - 1)
        nc.vector.memset(ald1[:], 0)
        nc.scalar.dma_start(
            out=ald1[:, :NBLK - 1, :],
            in_=table1[bass.ds(own0, nfull), D + H:D + 2 * H]
                .rearrange("(b p) c -> p b c", p=128))
        nc.scalar.dma_start(
            out=ald1[:g.LASTB, NBLK - 1, :],
            in_=table1[bass.ds(own0 + nfull, g.LASTB), D + H:D + 2 * H])

        # ---------------- edge phase (both layers) ---------------------------
        def edge_phase(tbl, aldst_sb, layer):
            ch_off = 0
            o16_lo = 0
            o16_hi = 0
            pool_ps = [None] * KC
            with tc.tile_pool(name=f"eg{layer}", bufs=3) as eg, \
                 tc.tile_pool(name=f"es{layer}", bufs=3) as es, \
                 tc.tile_pool(name=f"er{layer}", bufs=8) as er, \
                 tc.tile_pool(name=f"ep{layer}", bufs=2, space="PSUM") as epp, \
                 tc.tile_pool(name=f"ea{layer}", bufs=2, space="PSUM") as eap, \
                 tc.tile_pool(name=f"et{layer}", bufs=(2 if layer == 0 else 1), space="PSUM") as etp, \
                 tc.tile_pool(name=f"etr{layer}", bufs=2, space="PSUM") as etr:
                for b in range(NBLK):
                    Kl, Kh = K_lo[b], K_hi[b]
                    Kb = Kl + Kh
                    dodbg = DEBUG and layer == 0
                    S_all = es.tile([128, KMX, 128], BF16, tag="sall")
                    nc.vector.tensor_tensor(
                        out=S_all[:, :Kb, :],
                        in0=dcol_sb[:, ch_off:ch_off + Kb, None].to_broadcast([128, Kb, 128]),
                        in1=iota_rep[:, None, :].to_broadcast([128, Kb, 128]),
                        op=mybir.AluOpType.is_equal)
                    ST_all = es.tile([128, KMX, 128], BF16, tag="stall")
                    for j in range(Kb):
                        stp = etr.tile([128, 128], BF16, tag="stp")
                        nc.tensor.transpose(out=stp[:], in_=S_all[:, j, :],
                                            identity=identb[:])
                        nc.scalar.activation(out=ST_all[:, j, :], in_=stp[:],
                                             func=mybir.ActivationFunctionType.Copy)
                    gl = eg.tile([128, KMAXL, RPAD], BF16, tag="glo")
                    nc.gpsimd.dma_gather(
                        out_ap=gl[:, :Kl, :], in_ap=tbl[0:g.LOROWS, :],
                        idxs_ap=il_all[:, o16_lo:o16_lo + Kl * 8], num_idxs=Kl * 128,
                        num_idxs_reg=Kl * 128, elem_size=RPAD, single_packet=False)
                    gh = eg.tile([128, KMAXH, RPAD], BF16, tag="ghi")
                    nc.gpsimd.dma_gather(
                        out_ap=gh[:, :Kh, :], in_ap=tbl[g.LOROWS:g.N, :],
                        idxs_ap=ih_all[:, o16_hi:o16_hi + Kh * 8], num_idxs=Kh * 128,
                        num_idxs_reg=Kh * 128, elem_size=RPAD, single_packet=False)

                    if dodbg:
                        nc.sync.dma_start(out=dbg_S[:, ch_off:ch_off + Kb, :],
                                          in_=S_all[:, :Kb, :])
                        nc.sync.dma_start(out=dbg_ST[:, ch_off:ch_off + Kb, :],
                                          in_=ST_all[:, :Kb, :])
                    bps = epp.tile([128, D + H], F32, tag="bps")  # [msgs | den]
                    for j in range(Kb):
                        gt, jj = (gl, j) if j < Kl else (gh, j - Kl)
                        al_ps = eap.tile([128, H], F32, tag="alps")
                        nc.tensor.matmul(out=al_ps[:], lhsT=ST_all[:, j, :],
                                         rhs=aldst_sb[:, b, :], start=True, stop=True)
                        lg = er.tile([128, H], F32, tag="lg")
                        nc.vector.tensor_tensor(out=lg[:], in0=gt[:, jj, D:D + H],
                                                in1=al_ps[:], op=mybir.AluOpType.add)
                        nc.vector.scalar_tensor_tensor(
                            out=lg[:], in0=lg[:], scalar=NEG_SLOPE, in1=lg[:],
                            op0=mybir.AluOpType.mult, op1=mybir.AluOpType.max)
                        exf = er.tile([128, H], F32, tag="exf")
                        nc.scalar.activation(out=exf[:], in_=lg[:],
                                             func=mybir.ActivationFunctionType.Exp)
                        rhs = er.tile([128, D + H], BF16, tag="rhs")
                        hh = H // 2
                        nc.vector.tensor_tensor(
                            out=rhs[:, 0:hh * g.C].rearrange("p (h c) -> p h c", h=hh),
                            in0=gt[:, jj, 0:hh * g.C].rearrange("p (h c) -> p h c", h=hh),
                            in1=exf[:, 0:hh, None].to_broadcast([128, hh, g.C]),
                            op=mybir.AluOpType.mult)
                        for h in range(hh, H):
                            nc.scalar.activation(
                                out=rhs[:, h * g.C:(h + 1) * g.C],
                                in_=gt[:, jj, h * g.C:(h + 1) * g.C],
                                func=mybir.ActivationFunctionType.Copy,
                                scale=exf[:, h:h + 1])
                        nc.scalar.activation(out=rhs[:, D:D + H], in_=lg[:],
                                             func=mybir.ActivationFunctionType.Exp)
                        if dodbg:
                            nc.sync.dma_start(out=dbg_rhs[:, ch_off + j, :], in_=rhs[:])
                            alcp = er.tile([128, H], F32, tag="alcp")
                            nc.vector.tensor_copy(out=alcp[:], in_=al_ps[:])
                            nc.sync.dma_start(out=dbg_alps[:, ch_off + j, :], in_=alcp[:])
                            gcast = er.tile([128, RPAD], F32, tag="gcast")
                            nc.vector.tensor_copy(out=gcast[:], in_=gt[:, jj, :])
                            nc.sync.dma_start(out=dbg_g[:, ch_off + j, :], in_=gcast[:])
                        nc.tensor.matmul(out=bps[:], lhsT=S_all[:, j, :], rhs=rhs[:],
                                         start=(j == 0), stop=(j == Kb - 1))

                    den = er.tile([128, H], F32, tag="den")
                    nc.vector.tensor_scalar_add(den[:], bps[:, D:D + H], DEN_EPS)
                    rcp = er.tile([128, H], F32, tag="rcp")
                    nc.vector.reciprocal(rcp[:], den[:])
                    hv = er.tile([128, D], F32, tag="hv")
                    nc.vector.tensor_tensor(
                        out=hv[:].rearrange("p (h c) -> p h c", h=H),
                        in0=bps[:, 0:D].rearrange("p (h c) -> p h c", h=H),
                        in1=rcp[:, :, None].to_broadcast([128, H, g.C]),
                        op=mybir.AluOpType.mult)
                    bias = b0_sb if layer == 0 else b1_sb
                    nc.vector.tensor_tensor(out=hv[:], in0=hv[:], in1=bias[:],
                                            op=mybir.AluOpType.add)
                    if layer == 0:
                        um = er.tile([128, D], F32, tag="um")
                        nc.vector.tensor_scalar_min(um[:], hv[:], 0.0)
                        nc.scalar.activation(out=um[:], in_=um[:],
                                             func=mybir.ActivationFunctionType.Exp)
                        nc.vector.scalar_tensor_tensor(
                            out=hv[:], in0=hv[:], scalar=0.0, in1=um[:],
                            op0=mybir.AluOpType.max, op1=mybir.AluOpType.add)
                        nc.vector.tensor_scalar_add(hv[:], hv[:], -1.0)
                        if DEBUG:
                            nc.sync.dma_start(
                                out=dbg_h1[b * 128:b * 128 + sizes[b], :],
                                in_=hv[:sizes[b], :])
                        for kc in range(KC):
                            tps = etp.tile([128, 128], F32, tag="tps")
                            nc.tensor.transpose(
                                out=tps[:], in_=hv[:, kc * 128:(kc + 1) * 128],
                                identity=ident[:])
                            nc.vector.tensor_copy(out=stash[:, kc, b, :], in_=tps[:])
                    else:
                        if DEBUG:
                            nc.sync.dma_start(
                                out=dbg_h2[b * 128:b * 128 + sizes[b], :],
                                in_=hv[:sizes[b], :])
                        ov = ones_f if sizes[b] == 128 else ones_p
                        for kc in range(KC):
                            if pool_ps[kc] is None:
                                pps = etp.tile([128, 1], F32, tag=f"pool{kc}")
                                pool_ps[kc] = pps
                            nc.tensor.matmul(
                                out=pool_ps[kc][:],
                                lhsT=hv[:, kc * 128:(kc + 1) * 128],
                                rhs=ov[:], start=(b == 0), stop=(b == NBLK - 1))
                    ch_off += Kb
                    o16_lo += Kl * 8
                    o16_hi += Kh * 8
                if layer == 1:
                    for kc in range(KC):
                        nc.vector.tensor_copy(out=pool_sb[:, kc:kc + 1],
                                              in_=pool_ps[kc][:])

        with nc.named_scope("l1edge"):
            edge_phase(table1, ald1, 0)

        # ---------------- phase 3: L2 projection (own shard) -----------------
        ctx_l2 = nc.named_scope("l2proj"); ctx_l2.__enter__()
        with tc.tile_pool(name="l2", bufs=3) as l2, \
             tc.tile_pool(name="l2p", bufs=2, space="PSUM") as l2p:
            for b in range(NBLK):
                ps = l2p.tile([128, ROW], F32, tag="pp2")
                for kc in range(KC):
                    nc.tensor.matmul(out=ps[:], lhsT=stash[:, kc, b, :],
                                     rhs=w1_sb[:, kc, :],
                                     start=(kc == 0), stop=(kc == KC - 1))
                sb = l2.tile([128, ROW], BF16, tag="sb2")
                nc.vector.tensor_copy(out=sb[:sizes[b], :], in_=ps[:sizes[b], :])
                nc.sync.dma_start(out=l2loc[b * 128: b * 128 + sizes[b], :ROW],
                                  in_=sb[:sizes[b], :])

        ctx_l2.__exit__(None, None, None)
        nc.vector.memset(ald2[:], 0)
        nc.scalar.dma_start(
            out=ald2[:, :NBLK - 1, :],
            in_=l2loc[0:nfull, D + H:D + 2 * H].rearrange("(b p) c -> p b c", p=128))
        nc.scalar.dma_start(
            out=ald2[:g.LASTB, NBLK - 1, :],
            in_=l2loc[nfull:g.NPC, D + H:D + 2 * H])

        # ---------------- phase 4: AllGather L2 table ------------------------
        with nc.named_scope("allgather"):
            nc.gpsimd.collective_compute(
                "AllGather", mybir.AluOpType.bypass,
                replica_groups=[list(range(g.ncores))],
                ins=[l2loc[:]], outs=[table2[:]])

        # ---------------- phase 5: L2 edge + pool ----------------------------
        with nc.named_scope("l2edge"):
            edge_phase(table2, ald2, 1)

        if DEBUG:
            nc.sync.dma_start(out=dbg_dcol[:], in_=dcol_sb[:])
            nc.sync.dma_start(out=dbg_cst[:], in_=cst[:])
            nc.sync.dma_start(out=dbg_t1[:], in_=table1[:])
            nc.sync.dma_start(out=dbg_l2[:], in_=l2loc[:])
            nc.sync.dma_start(out=dbg_t2[:], in_=table2[:])
            nc.sync.dma_start(out=dbg_pool[:], in_=pool_sb[:])
            nc.sync.dma_start(out=dbg_ald[:], in_=ald1[:])

        # ---------------- phase 6: AllReduce + classifier --------------------
        nc.sync.dma_start(out=ar_in[:], in_=pool_sb[:])
        nc.gpsimd.collective_compute(
            "AllReduce", mybir.AluOpType.add,
            replica_groups=[list(range(g.ncores))],
            ins=[ar_in[:]], outs=[ar_out[:]])
        with tc.tile_pool(name="fin", bufs=1) as fin, \
             tc.tile_pool(name="finp", bufs=1, space="PSUM") as finp:
            pooled = fin.tile([128, KC], F32)
            nc.sync.dma_start(out=pooled[:], in_=ar_out[:])
            lgp = finp.tile([1, g.NCLS], F32, tag="lgp")
            for kc in range(KC):
                nc.tensor.matmul(out=lgp[:], lhsT=pooled[:, kc:kc + 1],
                                 rhs=clsw_sb[:, kc, :],
                                 start=(kc == 0), stop=(kc == KC - 1))
            lgs = fin.tile([1, g.NCLS], F32)
            nc.vector.tensor_scalar_mul(lgs[:], lgp[:], 1.0 / g.N)
            nc.vector.tensor_tensor(out=lgs[:], in0=lgs[:], in1=clsb_sb[:],
                                    op=mybir.AluOpType.add)
            nc.scalar.activation(out=lgs[:], in_=lgs[:],
                                 func=mybir.ActivationFunctionType.Exp)
            ssum = fin.tile([1, 1], F32)
            nc.vector.tensor_reduce(out=ssum[:], in_=lgs[:],
                                    axis=mybir.AxisListType.X, op=mybir.AluOpType.add)
            rs = fin.tile([1, 1], F32)
            nc.vector.reciprocal(rs[:], ssum[:])
            nc.vector.tensor_tensor(out=lgs[:], in0=lgs[:],
                                    in1=rs[:].to_broadcast([1, g.NCLS]),
                                    op=mybir.AluOpType.mult)
            nc.sync.dma_start(out=out_ext[:], in_=lgs[:])
    nc.finalize()
    return nc


# ----------------------------------------------------------------------------
# host entry
# ----------------------------------------------------------------------------

def _fold_weights(W, a_src, a_dst):
    H, C = a_src.shape
    D = W.shape[1]
    Asrc = np.zeros((D, H), np.float32)
    Adst = np.zeros((D, H), np.float32)
    for h in range(H):
        Asrc[h * C:(h + 1) * C, h] = a_src[h]
        Adst[h * C:(h + 1) * C, h] = a_dst[h]
    return np.concatenate([W, W @ Asrc, W @ Adst], axis=1)  # [in, D+2H]


def kernel(x, edge_index, W0, a0_src, a0_dst, b0, W1, a1_src, a1_dst, b1,
           cls_W, cls_b):
    x = np.asarray(x, np.float32)
    edge_index = np.asarray(edge_index, np.int64)
    N, F = x.shape
    a0_src = np.asarray(a0_src, np.float32)
    H, C = a0_src.shape
    cls_b = np.asarray(cls_b, np.float32)
    NCLS = cls_b.shape[0]
    g = Geo(N, F, H, C, NCLS)
    KC = g.D // 128

    pi, inv_pi, K_lo, K_hi, metas = preprocess(edge_index, g)

    bf = lambda a: np.ascontiguousarray(np.asarray(a, np.float32).astype(ml_dtypes.bfloat16))
    W0p = bf(_fold_weights(np.asarray(W0, np.float32), a0_src,
                           np.asarray(a0_dst, np.float32)))
    W1f = _fold_weights(np.asarray(W1, np.float32), np.asarray(a1_src, np.float32),
                        np.asarray(a1_dst, np.float32))
    W1p = bf(W1f.reshape(KC, 128, g.ROW).transpose(1, 0, 2))
    xTb = bf(x[inv_pi].T)

    consts = np.zeros((128, 259), np.float32)
    consts[:, 0] = np.arange(128)
    consts[:, 1:129] = np.eye(128)
    consts[:, 129] = 1.0
    consts[:g.LASTB, 130] = 1.0
    consts[:, 131:259] = np.arange(128)[None, :]
    clsWr = np.ascontiguousarray(
        np.asarray(cls_W, np.float32).reshape(KC, 128, NCLS).transpose(1, 0, 2))

    common = dict(
        xT=xTb, W0p=W0p, W1p=W1p,
        b0r=np.tile(np.asarray(b0, np.float32)[None, :], (128, 1)),
        b1r=np.tile(np.asarray(b1, np.float32)[None, :], (128, 1)),
        clsW=clsWr, clsb=cls_b[None, :],
        consts=consts,
    )
    in_maps = [dict(common, **metas[k]) for k in range(g.ncores)]

    nc = build_program(g, K_lo, K_hi)
    res = run_bass_kernel_spmd(nc, in_maps, list(range(g.ncores)))
    if DEBUG:
        kernel.last_debug = (res, pi, inv_pi, K_lo, K_hi, metas, g)
    return np.asarray(res.results[0]["out"], np.float32)



# revision 2
# speedup vs baseline: 1.3899x; 1.3899x over previous
"""Two-layer GAT (PyG GATConv-equivalent) on 8 Trainium2 NeuronCores.

Strategy (graph/data parallel, per the sharding hint):
  - Nodes are sharded by destination across the 8 cores (N/8 each).
  - Layer-1 projection x@W0 is computed replicated (x is replicated and the
    matmul is cheap), producing on every core a full node "table" in DRAM
    with 512-byte rows [h fp8e4 (256B) | al_src bf16 (8B) | al_dst bf16 (8B)
    | pad] so a single dma_gather per edge chunk fetches everything
    attention needs at the DMA sweet spot (512B rows avoid the <512B 2x
    descriptor latency penalty; fp8 h makes room for the al columns).
  - Edge phase: per 128-dst block, edges are gathered in 128-edge chunks
    round-robined over 4 SWDGE queues; exp(leakyrelu(al_src+al_dst)) is
    computed on-chip (no segment-max subtraction: logits are tiny for this
    weight scale, exp cannot overflow and softmax is shift-invariant),
    messages are ex-scaled and segment-summed into PSUM via a 0/1
    selector-matrix matmul; the softmax denominator rides along as 4 extra
    columns; a reciprocal-multiply normalizes at block end.
  - Layer-2 projection runs on each core's own shard only; one 8-core
    AllGather replicates the layer-2 table; the edge phase repeats;
    mean-pool partials are AllReduced; classifier + softmax run replicated.

Host-side work is limited to graph topology preprocessing (self-loops, dst
sharding, degree-balanced 128-dst blocks, int16 gather-index slabs) and
weight folding (concatenating W@a_src / W@a_dst columns onto W); all
O(N*D) model compute runs on device.
"""
import sys

for _p in ("/opt/trn_rl_repo", "/root/.axon_site/_ro/trn_rl_repo"):
    if _p not in sys.path:
        sys.path.append(_p)

import numpy as np
import ml_dtypes

import concourse.bass as bass
import concourse.tile as tile
from concourse import bacc, mybir
from concourse.bass_utils import run_bass_kernel_spmd

F32 = mybir.dt.float32
BF16 = mybir.dt.bfloat16
FP8 = mybir.dt.float8e4
I16 = mybir.dt.int16

NEG_SLOPE = 0.2
DEN_EPS = 1e-30
NQ = 4  # SWDGE queues for gathers


class Geo:
    def __init__(self, N, F, H, C, NCLS, ncores=8):
        self.N, self.F, self.H, self.C, self.NCLS = N, F, H, C, NCLS
        self.D = H * C
        self.ncores = ncores
        assert N % ncores == 0
        self.NPC = N // ncores               # nodes per core
        self.NBLK = (self.NPC + 127) // 128  # dst blocks per core
        self.LASTB = self.NPC - 128 * (self.NBLK - 1)  # rows in last block
        self.LOROWS = (N // 2 + 127) // 128 * 128      # lo/hi table split row
        assert self.LOROWS < 32768 and self.N - self.LOROWS <= 32768
        self.ROW = self.D + 2 * H            # proj psum cols [h|alsrc|aldst]
        self.RB = 512                        # table row bytes (fp8 cols)
        self.WB = self.D + 2 * H * 2         # useful row bytes (272)
        assert self.WB <= self.RB


def block_sizes(g):
    return [128] * (g.NBLK - 1) + [g.LASTB]


# ----------------------------------------------------------------------------
# host preprocessing (topology only)
# ----------------------------------------------------------------------------

def preprocess(edge_index, g):
    import heapq
    N, NPC, NBLK = g.N, g.NPC, g.NBLK
    src = np.concatenate([edge_index[0], np.arange(N, dtype=np.int64)])
    dst = np.concatenate([edge_index[1], np.arange(N, dtype=np.int64)])
    deg = np.bincount(dst, minlength=N)

    sizes = np.array(block_sizes(g))
    blk_of = np.empty(N, np.int32)
    pos_of = np.empty(N, np.int32)
    for k in range(g.ncores):
        nodes = np.arange(k * NPC, (k + 1) * NPC)
        order = nodes[np.argsort(-deg[nodes], kind="stable")]
        cnts = np.zeros(NBLK, np.int32)
        heap = [(0.0, b) for b in range(NBLK)]
        heapq.heapify(heap)
        for n in order:
            while True:
                s, b = heapq.heappop(heap)
                if cnts[b] < sizes[b]:
                    break
            blk_of[n] = b
            pos_of[n] = cnts[b]
            cnts[b] += 1
            if cnts[b] < sizes[b]:
                heapq.heappush(heap, (s + deg[n], b))
        assert (cnts == sizes).all()

    core = np.arange(N, dtype=np.int64) // NPC
    pi = core * NPC + blk_of.astype(np.int64) * 128 + pos_of
    inv_pi = np.empty(N, np.int64)
    inv_pi[pi] = np.arange(N)

    srcrow = pi[src]
    dcore = dst // NPC
    dblk = blk_of[dst].astype(np.int64)
    dpos = pos_of[dst]
    ishi = (srcrow >= g.LOROWS).astype(np.int64)

    key = (dcore * NBLK + dblk) * 2 + ishi
    order = np.argsort(key, kind="stable")
    skey = key[order]
    ssrc = srcrow[order]
    sdpos = dpos[order]
    nkey = g.ncores * NBLK * 2
    starts = np.searchsorted(skey, np.arange(nkey))
    ends = np.searchsorted(skey, np.arange(nkey) + 1)
    cnt = (ends - starts).reshape(g.ncores, NBLK, 2)

    K_lo = [max(1, int(np.ceil(cnt[:, b, 0].max() / 128))) for b in range(NBLK)]
    K_hi = [max(1, int(np.ceil(cnt[:, b, 1].max() / 128))) for b in range(NBLK)]

    metas = []
    for k in range(g.ncores):
        idx_lo, idx_hi, dcol = [], [], []
        for b in range(NBLK):
            for hi, Kb in ((0, K_lo[b]), (1, K_hi[b])):
                i0 = starts[(k * NBLK + b) * 2 + hi]
                i1 = ends[(k * NBLK + b) * 2 + hi]
                nn = Kb * 128
                rows = np.zeros(nn, np.int64)
                dl = np.full(nn, -1.0, np.float32)
                rows[: i1 - i0] = ssrc[i0:i1] - (g.LOROWS if hi else 0)
                dl[: i1 - i0] = sdpos[i0:i1]
                slab = np.tile(rows.reshape(-1, 16).T.astype(np.int16), (8, 1))
                (idx_hi if hi else idx_lo).append(slab)
                dcol.append(dl.reshape(Kb, 128).T)
        metas.append(dict(
            idx_lo=np.ascontiguousarray(np.concatenate(idx_lo, axis=1)),
            idx_hi=np.ascontiguousarray(np.concatenate(idx_hi, axis=1)),
            dstcol=np.ascontiguousarray(np.concatenate(dcol, axis=1), dtype=np.float32),
        ))
    return pi, inv_pi, K_lo, K_hi, metas


# ----------------------------------------------------------------------------
# device program
# ----------------------------------------------------------------------------

def build_program(g, K_lo, K_hi):
    NBLK, RB, D, H = g.NBLK, g.RB, g.D, g.H
    ROW, WB = g.ROW, g.WB
    NT16_lo = sum(K_lo) * 8
    NT16_hi = sum(K_hi) * 8
    NCH = sum(K_lo) + sum(K_hi)
    KMAXL, KMAXH = max(K_lo), max(K_hi)
    KMX = KMAXL + KMAXH
    ntile = (g.N + 127) // 128
    sizes = block_sizes(g)
    KC = D // 128   # feature 128-chunks (2)

    nc = bacc.Bacc(None, target_bir_lowering=False, num_swdge_queues=NQ)
    dp = lambda n, s, d: nc.declare_dram_parameter(n, s, d, isOutput=False)
    xT = dp("xT", [g.F, g.N], BF16)
    W0p = dp("W0p", [g.F, ROW], BF16)
    W1p = dp("W1p", [128, KC, ROW], BF16)
    b0r = dp("b0r", [128, D], F32)
    b1r = dp("b1r", [128, D], F32)
    clsW = dp("clsW", [128, KC, g.NCLS], F32)
    clsb = dp("clsb", [1, g.NCLS], F32)
    idx_lo = dp("idx_lo", [128, NT16_lo], I16)
    idx_hi = dp("idx_hi", [128, NT16_hi], I16)
    dstcol = dp("dstcol", [128, NCH], F32)
    # consts: [iota_col | identity(128) | ones | ones_partial | iota_row(row0)]
    consts = dp("consts", [128, 259], F32)
    out_ext = nc.declare_dram_parameter("out", [1, g.NCLS], F32, isOutput=True)

    table1 = nc.dram_tensor("table1", [g.N, RB], FP8)
    l2loc = nc.dram_tensor("l2loc", [g.NPC, RB], FP8)
    table2 = nc.dram_tensor("table2", [g.N, RB], FP8, addr_space="Shared")
    ar_in = nc.dram_tensor("ar_in", [128, KC], F32)
    ar_out = nc.dram_tensor("ar_out", [128, KC], F32, addr_space="Shared")

    with tile.TileContext(nc) as tc:
      with tc.tile_pool(name="res", bufs=1) as res:
        cst = res.tile([128, 259], F32)
        nc.scalar.dma_start(out=cst[:], in_=consts[:])
        iota_col = cst[:, 0:1]
        ident = cst[:, 1:129]
        ones_f = cst[:, 129:130]
        ones_p = cst[:, 130:131]
        iota_rep = cst[:, 131:259]
        dcol_sb = res.tile([128, NCH], F32)
        nc.scalar.dma_start(out=dcol_sb[:], in_=dstcol[:])
        b0_sb = res.tile([128, D], F32)
        nc.scalar.dma_start(out=b0_sb[:], in_=b0r[:])
        b1_sb = res.tile([128, D], F32)
        nc.scalar.dma_start(out=b1_sb[:], in_=b1r[:])
        w1_sb = res.tile([128, KC, ROW], BF16)
        nc.scalar.dma_start(out=w1_sb[:], in_=W1p[:])
        w0_sb = res.tile([g.F, ROW], BF16)
        nc.scalar.dma_start(out=w0_sb[:], in_=W0p[:])
        clsw_sb = res.tile([128, KC, g.NCLS], F32)
        nc.scalar.dma_start(out=clsw_sb[:], in_=clsW[:])
        clsb_sb = res.tile([1, g.NCLS], F32)
        nc.scalar.dma_start(out=clsb_sb[:], in_=clsb[:])
        il_all = res.tile([128, NT16_lo], I16)
        nc.scalar.dma_start(out=il_all[:], in_=idx_lo[:])
        ih_all = res.tile([128, NT16_hi], I16)
        nc.scalar.dma_start(out=ih_all[:], in_=idx_hi[:])
        identb = res.tile([128, 128], BF16)
        nc.vector.tensor_copy(out=identb[:], in_=ident[:])
        stash = res.tile([128, KC, NBLK, 128], BF16)   # h1^T for L2 projection
        ald1 = res.tile([128, NBLK, H], BF16)
        ald2 = res.tile([128, NBLK, H], BF16)
        pool_sb = res.tile([128, KC], F32)

        # ---------------- phase 1: L1 projection (replicated, full N) -------
        ctx_pj = nc.named_scope("l1proj"); ctx_pj.__enter__()
        with tc.tile_pool(name="pj", bufs=3) as pj, \
             tc.tile_pool(name="pjp", bufs=2, space="PSUM") as pjp:
            GRP = 4
            t = 0
            while t < ntile:
                gn = min(GRP, ntile - t)
                r0 = t * 128
                rows = min(g.N, r0 + gn * 128) - r0
                full = [max(0, min(128, rows - i * 128)) for i in range(gn)]
                xt_sb = pj.tile([g.F, GRP, 128], BF16, tag="xt")
                nc.scalar.dma_start(
                    out=xt_sb[:, :gn, :].rearrange("p g n -> p (g n)")[:, :rows],
                    in_=xT[:, r0:r0 + rows])
                st_sb = pj.tile([128, GRP, RB], FP8, tag="st")
                for i in range(gn):
                    if full[i] == 0:
                        continue
                    ps = pjp.tile([128, ROW], F32, tag="pp")
                    nc.tensor.matmul(out=ps[:full[i], :], lhsT=xt_sb[:, i, :full[i]],
                                     rhs=w0_sb[:], start=True, stop=True)
                    nc.vector.tensor_copy(out=st_sb[:full[i], i, 0:D],
                                          in_=ps[:full[i], 0:D])
                    nc.scalar.activation(
                        out=st_sb[:full[i], i, D:WB].bitcast(BF16),
                        in_=ps[:full[i], D:ROW],
                        func=mybir.ActivationFunctionType.Copy)
                if rows == gn * 128:
                    nc.sync.dma_start(
                        out=table1[r0:r0 + rows, :WB].rearrange("(i p) c -> p i c", p=128),
                        in_=st_sb[:, :gn, :WB])
                else:
                    for i in range(gn):
                        if full[i]:
                            nc.sync.dma_start(
                                out=table1[r0 + i * 128: r0 + i * 128 + full[i], :WB],
                                in_=st_sb[:full[i], i, :WB])
                t += gn

        ctx_pj.__exit__(None, None, None)
        # aldst slab for layer 1: own-shard rows of the replicated table1
        pid = nc.partition_id()
        own0 = pid * g.NPC
        nfull = 128 * (NBLK - 1)
        nc.vector.memset(ald1[:], 0)
        nc.scalar.dma_start(
            out=ald1[:, :NBLK - 1, :],
            in_=table1[bass.ds(own0, nfull), D + 2 * H:WB].bitcast(BF16)
                .rearrange("(b p) c -> p b c", p=128))
        nc.scalar.dma_start(
            out=ald1[:g.LASTB, NBLK - 1, :],
            in_=table1[bass.ds(own0 + nfull, g.LASTB), D + 2 * H:WB].bitcast(BF16))

        # ---------------- edge phase (both layers) ---------------------------
        def edge_phase(tbl, aldst_sb, layer):
            ch_off = 0
            o16_lo = 0
            o16_hi = 0
            qn = 0
            pool_ps = [None] * KC
            with tc.tile_pool(name=f"eg{layer}", bufs=3) as eg, \
                 tc.tile_pool(name=f"es{layer}", bufs=3) as es, \
                 tc.tile_pool(name=f"er{layer}", bufs=8) as er, \
                 tc.tile_pool(name=f"ep{layer}", bufs=2, space="PSUM") as epp, \
                 tc.tile_pool(name=f"ea{layer}", bufs=2, space="PSUM") as eap, \
                 tc.tile_pool(name=f"et{layer}", bufs=(2 if layer == 0 else 1), space="PSUM") as etp, \
                 tc.tile_pool(name=f"etr{layer}", bufs=2, space="PSUM") as etr:
                for b in range(NBLK):
                    Kl, Kh = K_lo[b], K_hi[b]
                    Kb = Kl + Kh
                    S_all = es.tile([128, KMX, 128], BF16, tag="sall")
                    nc.vector.tensor_tensor(
                        out=S_all[:, :Kb, :],
                        in0=dcol_sb[:, ch_off:ch_off + Kb, None].to_broadcast([128, Kb, 128]),
                        in1=iota_rep[:, None, :].to_broadcast([128, Kb, 128]),
                        op=mybir.AluOpType.is_equal)
                    ST_all = es.tile([128, KMX, 128], BF16, tag="stall")
                    for j in range(Kb):
                        stp = etr.tile([128, 128], BF16, tag="stp")
                        nc.tensor.transpose(out=stp[:], in_=S_all[:, j, :],
                                            identity=identb[:])
                        nc.scalar.activation(out=ST_all[:, j, :], in_=stp[:],
                                             func=mybir.ActivationFunctionType.Copy)
                    gl = eg.tile([128, KMAXL, RB], FP8, tag="glo")
                    nc.gpsimd.dma_gather(
                        out_ap=gl[:, :Kl, :], in_ap=tbl[0:g.LOROWS, :],
                        idxs_ap=il_all[:, o16_lo:o16_lo + Kl * 8], num_idxs=Kl * 128,
                        num_idxs_reg=Kl * 128, elem_size=RB, single_packet=False,
                        queue_num=qn % NQ)
                    gh = eg.tile([128, KMAXH, RB], FP8, tag="ghi")
                    nc.gpsimd.dma_gather(
                        out_ap=gh[:, :Kh, :], in_ap=tbl[g.LOROWS:g.N, :],
                        idxs_ap=ih_all[:, o16_hi:o16_hi + Kh * 8], num_idxs=Kh * 128,
                        num_idxs_reg=Kh * 128, elem_size=RB, single_packet=False,
                        queue_num=(qn + 1) % NQ)
                    qn += 2

                    bps = epp.tile([128, D + H], F32, tag="bps")  # [msgs | den]
                    for j in range(Kb):
                        gt, jj = (gl, j) if j < Kl else (gh, j - Kl)
                        gal = gt[:, jj, D:WB].bitcast(BF16)  # [128, 8] = [as|ad]
                        al_ps = eap.tile([128, H], F32, tag="alps")
                        nc.tensor.matmul(out=al_ps[:], lhsT=ST_all[:, j, :],
                                         rhs=aldst_sb[:, b, :], start=True, stop=True)
                        lg = er.tile([128, H], F32, tag="lg")
                        nc.vector.tensor_tensor(out=lg[:], in0=gal[:, 0:H],
                                                in1=al_ps[:], op=mybir.AluOpType.add)
                        nc.vector.scalar_tensor_tensor(
                            out=lg[:], in0=lg[:], scalar=NEG_SLOPE, in1=lg[:],
                            op0=mybir.AluOpType.mult, op1=mybir.AluOpType.max)
                        rhs = er.tile([128, D + H], BF16, tag="rhs")
                        nc.scalar.activation(out=rhs[:, D:D + H], in_=lg[:],
                                             func=mybir.ActivationFunctionType.Exp)
                        nc.vector.tensor_tensor(
                            out=rhs[:, 0:D].rearrange("p (h c) -> p h c", h=H),
                            in0=gt[:, jj, 0:D].rearrange("p (h c) -> p h c", h=H),
                            in1=rhs[:, D:D + H][:, :, None].to_broadcast([128, H, g.C]),
                            op=mybir.AluOpType.mult)
                        nc.tensor.matmul(out=bps[:], lhsT=S_all[:, j, :], rhs=rhs[:],
                                         start=(j == 0), stop=(j == Kb - 1))

                    den = er.tile([128, H], F32, tag="den")
                    nc.vector.tensor_scalar_add(den[:], bps[:, D:D + H], DEN_EPS)
                    rcp = er.tile([128, H], F32, tag="rcp")
                    nc.vector.reciprocal(rcp[:], den[:])
                    hv = er.tile([128, D], F32, tag="hv")
                    nc.vector.tensor_tensor(
                        out=hv[:].rearrange("p (h c) -> p h c", h=H),
                        in0=bps[:, 0:D].rearrange("p (h c) -> p h c", h=H),
                        in1=rcp[:, :, None].to_broadcast([128, H, g.C]),
                        op=mybir.AluOpType.mult)
                    bias = b0_sb if layer == 0 else b1_sb
                    nc.vector.tensor_tensor(out=hv[:], in0=hv[:], in1=bias[:],
                                            op=mybir.AluOpType.add)
                    if layer == 0:
                        um = er.tile([128, D], F32, tag="um")
                        nc.vector.tensor_scalar_min(um[:], hv[:], 0.0)
                        nc.scalar.activation(out=um[:], in_=um[:],
                                             func=mybir.ActivationFunctionType.Exp)
                        nc.vector.scalar_tensor_tensor(
                            out=hv[:], in0=hv[:], scalar=0.0, in1=um[:],
                            op0=mybir.AluOpType.max, op1=mybir.AluOpType.add)
                        nc.vector.tensor_scalar_add(hv[:], hv[:], -1.0)
                        for kc in range(KC):
                            tps = etp.tile([128, 128], F32, tag="tps")
                            nc.tensor.transpose(
                                out=tps[:], in_=hv[:, kc * 128:(kc + 1) * 128],
                                identity=ident[:])
                            nc.vector.tensor_copy(out=stash[:, kc, b, :], in_=tps[:])
                    else:
                        ov = ones_f if sizes[b] == 128 else ones_p
                        for kc in range(KC):
                            if pool_ps[kc] is None:
                                pps = etp.tile([128, 1], F32, tag=f"pool{kc}")
                                pool_ps[kc] = pps
                            nc.tensor.matmul(
                                out=pool_ps[kc][:],
                                lhsT=hv[:, kc * 128:(kc + 1) * 128],
                                rhs=ov[:], start=(b == 0), stop=(b == NBLK - 1))
                    ch_off += Kb
                    o16_lo += Kl * 8
                    o16_hi += Kh * 8
                if layer == 1:
                    for kc in range(KC):
                        nc.vector.tensor_copy(out=pool_sb[:, kc:kc + 1],
                                              in_=pool_ps[kc][:])

        with nc.named_scope("l1edge"):
            edge_phase(table1, ald1, 0)

        # ---------------- phase 3: L2 projection (own shard) -----------------
        ctx_l2 = nc.named_scope("l2proj"); ctx_l2.__enter__()
        with tc.tile_pool(name="l2", bufs=3) as l2, \
             tc.tile_pool(name="l2p", bufs=2, space="PSUM") as l2p:
            for b in range(NBLK):
                ps = l2p.tile([128, ROW], F32, tag="pp2")
                for kc in range(KC):
                    nc.tensor.matmul(out=ps[:], lhsT=stash[:, kc, b, :],
                                     rhs=w1_sb[:, kc, :],
                                     start=(kc == 0), stop=(kc == KC - 1))
                sb = l2.tile([128, RB], FP8, tag="sb2")
                nc.vector.tensor_copy(out=sb[:sizes[b], 0:D], in_=ps[:sizes[b], 0:D])
                nc.scalar.activation(
                    out=sb[:sizes[b], D:WB].bitcast(BF16),
                    in_=ps[:sizes[b], D:ROW],
                    func=mybir.ActivationFunctionType.Copy)
                nc.sync.dma_start(out=l2loc[b * 128: b * 128 + sizes[b], :WB],
                                  in_=sb[:sizes[b], :WB])

        ctx_l2.__exit__(None, None, None)
        nc.vector.memset(ald2[:], 0)
        nc.scalar.dma_start(
            out=ald2[:, :NBLK - 1, :],
            in_=l2loc[0:nfull, D + 2 * H:WB].bitcast(BF16)
                .rearrange("(b p) c -> p b c", p=128))
        nc.scalar.dma_start(
            out=ald2[:g.LASTB, NBLK - 1, :],
            in_=l2loc[nfull:g.NPC, D + 2 * H:WB].bitcast(BF16))

        # ---------------- phase 4: AllGather L2 table ------------------------
        with nc.named_scope("allgather"):
            nc.gpsimd.collective_compute(
                "AllGather", mybir.AluOpType.bypass,
                replica_groups=[list(range(g.ncores))],
                ins=[l2loc[:]], outs=[table2[:]])

        # ---------------- phase 5: L2 edge + pool ----------------------------
        with nc.named_scope("l2edge"):
            edge_phase(table2, ald2, 1)

        # ---------------- phase 6: AllReduce + classifier --------------------
        nc.sync.dma_start(out=ar_in[:], in_=pool_sb[:])
        nc.gpsimd.collective_compute(
            "AllReduce", mybir.AluOpType.add,
            replica_groups=[list(range(g.ncores))],
            ins=[ar_in[:]], outs=[ar_out[:]])
        with tc.tile_pool(name="fin", bufs=1) as fin, \
             tc.tile_pool(name="finp", bufs=1, space="PSUM") as finp:
            pooled = fin.tile([128, KC], F32)
            nc.sync.dma_start(out=pooled[:], in_=ar_out[:])
            lgp = finp.tile([1, g.NCLS], F32, tag="lgp")
            for kc in range(KC):
                nc.tensor.matmul(out=lgp[:], lhsT=pooled[:, kc:kc + 1],
                                 rhs=clsw_sb[:, kc, :],
                                 start=(kc == 0), stop=(kc == KC - 1))
            lgs = fin.tile([1, g.NCLS], F32)
            nc.vector.tensor_scalar_mul(lgs[:], lgp[:], 1.0 / g.N)
            nc.vector.tensor_tensor(out=lgs[:], in0=lgs[:], in1=clsb_sb[:],
                                    op=mybir.AluOpType.add)
            nc.scalar.activation(out=lgs[:], in_=lgs[:],
                                 func=mybir.ActivationFunctionType.Exp)
            ssum = fin.tile([1, 1], F32)
            nc.vector.tensor_reduce(out=ssum[:], in_=lgs[:],
                                    axis=mybir.AxisListType.X, op=mybir.AluOpType.add)
            rs = fin.tile([1, 1], F32)
            nc.vector.reciprocal(rs[:], ssum[:])
            nc.vector.tensor_tensor(out=lgs[:], in0=lgs[:],
                                    in1=rs[:].to_broadcast([1, g.NCLS]),
                                    op=mybir.AluOpType.mult)
            nc.sync.dma_start(out=out_ext[:], in_=lgs[:])
    nc.finalize()
    return nc


# ----------------------------------------------------------------------------
# host entry
# ----------------------------------------------------------------------------

def _fold_weights(W, a_src, a_dst):
    H, C = a_src.shape
    D = W.shape[1]
    Asrc = np.zeros((D, H), np.float32)
    Adst = np.zeros((D, H), np.float32)
    for h in range(H):
        Asrc[h * C:(h + 1) * C, h] = a_src[h]
        Adst[h * C:(h + 1) * C, h] = a_dst[h]
    return np.concatenate([W, W @ Asrc, W @ Adst], axis=1)  # [in, D+2H]


def kernel(x, edge_index, W0, a0_src, a0_dst, b0, W1, a1_src, a1_dst, b1,
           cls_W, cls_b):
    x = np.asarray(x, np.float32)
    edge_index = np.asarray(edge_index, np.int64)
    N, F = x.shape
    a0_src = np.asarray(a0_src, np.float32)
    H, C = a0_src.shape
    cls_b = np.asarray(cls_b, np.float32)
    NCLS = cls_b.shape[0]
    g = Geo(N, F, H, C, NCLS)
    KC = g.D // 128

    pi, inv_pi, K_lo, K_hi, metas = preprocess(edge_index, g)

    bf = lambda a: np.ascontiguousarray(np.asarray(a, np.float32).astype(ml_dtypes.bfloat16))
    W0p = bf(_fold_weights(np.asarray(W0, np.float32), a0_src,
                           np.asarray(a0_dst, np.float32)))
    W1f = _fold_weights(np.asarray(W1, np.float32), np.asarray(a1_src, np.float32),
                        np.asarray(a1_dst, np.float32))
    W1p = bf(W1f.reshape(KC, 128, g.ROW).transpose(1, 0, 2))
    xTb = bf(x[inv_pi].T)

    consts = np.zeros((128, 259), np.float32)
    consts[:, 0] = np.arange(128)
    consts[:, 1:129] = np.eye(128)
    consts[:, 129] = 1.0
    consts[:g.LASTB, 130] = 1.0
    consts[:, 131:259] = np.arange(128)[None, :]
    clsWr = np.ascontiguousarray(
        np.asarray(cls_W, np.float32).reshape(KC, 128, NCLS).transpose(1, 0, 2))

    common = dict(
        xT=xTb, W0p=W0p, W1p=W1p,
        b0r=np.tile(np.asarray(b0, np.float32)[None, :], (128, 1)),
        b1r=np.tile(np.asarray(b1, np.float32)[None, :], (128, 1)),
        clsW=clsWr, clsb=cls_b[None, :],
        consts=consts,
    )
    in_maps = [dict(common, **metas[k]) for k in range(g.ncores)]

    nc = build_program(g, K_lo, K_hi)
    res = run_bass_kernel_spmd(nc, in_maps, list(range(g.ncores)))
    return np.asarray(res.results[0]["out"], np.float32)


# revision 11
# speedup vs baseline: 19.1532x; 13.7806x over previous
"""Two-layer GAT (PyG GATConv-equivalent) on 8 Trainium2 NeuronCores.

Strategy (graph/data parallel, per the sharding hint):
  - Nodes are sharded by destination across the 8 cores (N/8 each).
  - Layer-1 projection x@W0 is computed replicated (x is replicated and the
    matmul is cheap), producing on every core a full node "table" in DRAM
    with 512-byte rows [h fp8e4 (256B) | al_src bf16 (8B) | al_dst bf16 (8B)
    | pad] so a single dma_gather per edge chunk fetches everything
    attention needs at the DMA sweet spot (512B rows avoid the <512B 2x
    descriptor latency penalty; fp8 h makes room for the al columns).
  - Edge phase: per 128-dst block, the appended self-loop edges are loaded
    densely (their table rows are contiguous by construction) and the real
    edges are gathered in 128-edge chunks round-robined over 4 SWDGE queues
    with -1 index padding (the gather ucode trims trailing negatives, so
    per-core padding costs no descriptors); exp(leakyrelu(al_src+al_dst))
    runs batched per block (vector add + scalar Lrelu); messages are
    ex-scaled (heads split 3:1 across vector/scalar) and segment-summed
    into PSUM via a 0/1 selector-matrix matmul; the softmax denominator
    rides along as 4 extra columns; a reciprocal-multiply normalizes at
    block end.
  - Layer-2 projection runs on each core's own shard only; one 8-core
    AllGather replicates the layer-2 table; the edge phase repeats;
    mean-pool partials are AllReduced; classifier + softmax run replicated.

Host-side work is limited to graph topology preprocessing (self-loops, dst
sharding, degree-balanced 128-dst blocks, int16 gather-index slabs) and
weight folding (concatenating W@a_src / W@a_dst columns onto W); all
O(N*D) model compute runs on device.
"""
import sys

for _p in ("/opt/trn_rl_repo", "/root/.axon_site/_ro/trn_rl_repo"):
    if _p not in sys.path:
        sys.path.append(_p)

import numpy as np
import ml_dtypes

import concourse.bass as bass
import concourse.tile as tile
from concourse import bacc, mybir
from concourse.bass_utils import run_bass_kernel_spmd

F32 = mybir.dt.float32
BF16 = mybir.dt.bfloat16
FP8 = mybir.dt.float8e4
I16 = mybir.dt.int16

NEG_SLOPE = 0.2
NQ = 4  # SWDGE queues for gathers


class Geo:
    def __init__(self, N, F, H, C, NCLS, ncores=8):
        self.N, self.F, self.H, self.C, self.NCLS = N, F, H, C, NCLS
        self.D = H * C
        self.ncores = ncores
        assert N % ncores == 0
        self.NPC = N // ncores               # nodes per core
        self.NBLK = (self.NPC + 127) // 128  # dst blocks per core
        self.LASTB = self.NPC - 128 * (self.NBLK - 1)  # rows in last block
        self.LOROWS = (N // 2 + 127) // 128 * 128      # lo/hi table split row
        assert self.LOROWS < 32768 and self.N - self.LOROWS <= 32768
        self.ROW = self.D + 2 * H            # proj psum cols [h|alsrc|aldst]
        self.RB = 512                        # table row bytes (fp8 cols)
        self.WB = self.D + 2 * H * 2         # useful row bytes (272)
        assert self.WB <= self.RB


def block_sizes(g):
    return [128] * (g.NBLK - 1) + [g.LASTB]


# ----------------------------------------------------------------------------
# host preprocessing (topology only)
# ----------------------------------------------------------------------------

def preprocess(edge_index, g):
    import heapq
    N, NPC, NBLK = g.N, g.NPC, g.NBLK
    # appended self-loops are handled as a dense per-block chunk on device;
    # only the real edges go through the gather path (original u->u edges
    # included: they are distinct from the appended loops and both count).
    src = edge_index[0]
    dst = edge_index[1]
    deg = np.bincount(dst, minlength=N) + 1  # +1: the appended self-loop

    sizes = np.array(block_sizes(g))
    blk_of = np.empty(N, np.int32)
    pos_of = np.empty(N, np.int32)
    for k in range(g.ncores):
        nodes = np.arange(k * NPC, (k + 1) * NPC)
        order = nodes[np.argsort(-deg[nodes], kind="stable")]
        cnts = np.zeros(NBLK, np.int32)
        heap = [(0.0, b) for b in range(NBLK)]
        heapq.heapify(heap)
        for n in order:
            while True:
                s, b = heapq.heappop(heap)
                if cnts[b] < sizes[b]:
                    break
            blk_of[n] = b
            pos_of[n] = cnts[b]
            cnts[b] += 1
            if cnts[b] < sizes[b]:
                heapq.heappush(heap, (s + deg[n], b))
        assert (cnts == sizes).all()

    core = np.arange(N, dtype=np.int64) // NPC
    pi = core * NPC + blk_of.astype(np.int64) * 128 + pos_of
    inv_pi = np.empty(N, np.int64)
    inv_pi[pi] = np.arange(N)

    srcrow = pi[src]
    dcore = dst // NPC
    dblk = blk_of[dst].astype(np.int64)
    dpos = pos_of[dst]
    ishi = (srcrow >= g.LOROWS).astype(np.int64)

    key = (dcore * NBLK + dblk) * 2 + ishi
    order = np.argsort(key, kind="stable")
    skey = key[order]
    ssrc = srcrow[order]
    sdpos = dpos[order]
    nkey = g.ncores * NBLK * 2
    starts = np.searchsorted(skey, np.arange(nkey))
    ends = np.searchsorted(skey, np.arange(nkey) + 1)
    cnt = (ends - starts).reshape(g.ncores, NBLK, 2)

    K_lo = [max(1, int(np.ceil(cnt[:, b, 0].max() / 128))) for b in range(NBLK)]
    K_hi = [max(1, int(np.ceil(cnt[:, b, 1].max() / 128))) for b in range(NBLK)]

    self_col = np.full(128, -1.0, np.float32)
    metas = []
    for k in range(g.ncores):
        idx_lo, idx_hi, dcol = [], [], []
        for b in range(NBLK):
            sc = self_col.copy()
            sc[:sizes[b]] = np.arange(sizes[b])
            dcol.append(sc[:, None])
            for hi, Kb in ((0, K_lo[b]), (1, K_hi[b])):
                i0 = starts[(k * NBLK + b) * 2 + hi]
                i1 = ends[(k * NBLK + b) * 2 + hi]
                nn = Kb * 128
                rows = np.zeros(nn, np.int64)
                dl = np.full(nn, -1.0, np.float32)
                rows[: i1 - i0] = ssrc[i0:i1] - (g.LOROWS if hi else 0)
                dl[: i1 - i0] = sdpos[i0:i1]
                slab = np.tile(rows.reshape(-1, 16).T.astype(np.int16), (8, 1))
                (idx_hi if hi else idx_lo).append(slab)
                dcol.append(dl.reshape(Kb, 128).T)
        metas.append(dict(
            idx_lo=np.ascontiguousarray(np.concatenate(idx_lo, axis=1)),
            idx_hi=np.ascontiguousarray(np.concatenate(idx_hi, axis=1)),
            dstcol=np.ascontiguousarray(np.concatenate(dcol, axis=1), dtype=np.float32),
        ))
    return pi, inv_pi, K_lo, K_hi, metas


# ----------------------------------------------------------------------------
# device program
# ----------------------------------------------------------------------------

def build_program(g, K_lo, K_hi):
    NBLK, RB, D, H = g.NBLK, g.RB, g.D, g.H
    ROW, WB = g.ROW, g.WB
    NT16_lo = sum(K_lo) * 8
    NT16_hi = sum(K_hi) * 8
    NCH = NBLK + sum(K_lo) + sum(K_hi)   # +NBLK: dense self chunk per block
    KMAXL, KMAXH = max(K_lo), max(K_hi)
    KMXT = 1 + KMAXL + KMAXH
    ntile = (g.N + 127) // 128
    sizes = block_sizes(g)
    KC = D // 128   # feature 128-chunks (2)

    nc = bacc.Bacc(None, target_bir_lowering=False, num_swdge_queues=NQ)
    dp = lambda n, s, d: nc.declare_dram_parameter(n, s, d, isOutput=False)
    xT = dp("xT", [g.F, g.N], BF16)
    W0p = dp("W0p", [g.F, ROW], BF16)
    W1p = dp("W1p", [128, KC, ROW], BF16)
    b0r = dp("b0r", [128, D], F32)
    b1r = dp("b1r", [128, D], F32)
    clsW = dp("clsW", [128, KC, g.NCLS], F32)
    clsb = dp("clsb", [1, g.NCLS], F32)
    idx_lo = dp("idx_lo", [128, NT16_lo], I16)
    idx_hi = dp("idx_hi", [128, NT16_hi], I16)
    dstcol = dp("dstcol", [128, NCH], F32)
    # consts: [iota_col | identity(128) | ones | ones_partial | iota_row | zero]
    consts = dp("consts", [128, 260], F32)
    out_ext = nc.declare_dram_parameter("out", [1, g.NCLS], F32, isOutput=True)

    table1 = nc.dram_tensor("table1", [g.N, RB], FP8)
    l2loc = nc.dram_tensor("l2loc", [g.NPC, RB], FP8)
    table2 = nc.dram_tensor("table2", [g.N, RB], FP8, addr_space="Shared")
    ar_in = nc.dram_tensor("ar_in", [128, KC], F32)
    ar_out = nc.dram_tensor("ar_out", [128, KC], F32, addr_space="Shared")

    with tile.TileContext(nc) as tc:
      with tc.tile_pool(name="res", bufs=1) as res:
        cst = res.tile([128, 260], F32)
        nc.scalar.dma_start(out=cst[:], in_=consts[:])
        iota_col = cst[:, 0:1]
        ident = cst[:, 1:129]
        ones_f = cst[:, 129:130]
        ones_p = cst[:, 130:131]
        iota_rep = cst[:, 131:259]
        zero_c = cst[:, 259:260]
        dcol_sb = res.tile([128, NCH], F32)
        nc.scalar.dma_start(out=dcol_sb[:], in_=dstcol[:])
        b0_sb = res.tile([128, D], F32)
        nc.scalar.dma_start(out=b0_sb[:], in_=b0r[:])
        b1_sb = res.tile([128, D], F32)
        nc.scalar.dma_start(out=b1_sb[:], in_=b1r[:])
        w1_sb = res.tile([128, KC, ROW], BF16)
        nc.scalar.dma_start(out=w1_sb[:], in_=W1p[:])
        w0_sb = res.tile([g.F, ROW], BF16)
        nc.scalar.dma_start(out=w0_sb[:], in_=W0p[:])
        clsw_sb = res.tile([128, KC, g.NCLS], F32)
        nc.scalar.dma_start(out=clsw_sb[:], in_=clsW[:])
        clsb_sb = res.tile([1, g.NCLS], F32)
        nc.scalar.dma_start(out=clsb_sb[:], in_=clsb[:])
        il_all = res.tile([128, NT16_lo], I16)
        nc.scalar.dma_start(out=il_all[:], in_=idx_lo[:])
        ih_all = res.tile([128, NT16_hi], I16)
        nc.scalar.dma_start(out=ih_all[:], in_=idx_hi[:])
        identb = res.tile([128, 128], BF16)
        nc.vector.tensor_copy(out=identb[:], in_=ident[:])
        stash = res.tile([128, KC, NBLK, 128], BF16)   # h1^T for L2 projection
        ald1 = res.tile([128, NBLK, H], BF16)
        ald2 = res.tile([128, NBLK, H], BF16)
        pool_sb = res.tile([128, KC], F32)

        # ---------------- phase 1: L1 projection (replicated, full N) -------
        ctx_pj = nc.named_scope("l1proj"); ctx_pj.__enter__()
        with tc.tile_pool(name="pj", bufs=3) as pj, \
             tc.tile_pool(name="pjp", bufs=2, space="PSUM") as pjp:
            GRP = 4
            t = 0
            while t < ntile:
                gn = min(GRP, ntile - t)
                r0 = t * 128
                rows = min(g.N, r0 + gn * 128) - r0
                full = [max(0, min(128, rows - i * 128)) for i in range(gn)]
                xt_sb = pj.tile([g.F, GRP, 128], BF16, tag="xt")
                nc.scalar.dma_start(
                    out=xt_sb[:, :gn, :].rearrange("p g n -> p (g n)")[:, :rows],
                    in_=xT[:, r0:r0 + rows])
                st_sb = pj.tile([128, GRP, RB], FP8, tag="st")
                for i in range(gn):
                    if full[i] == 0:
                        continue
                    ps = pjp.tile([128, ROW], F32, tag="pp")
                    nc.tensor.matmul(out=ps[:full[i], :], lhsT=xt_sb[:, i, :full[i]],
                                     rhs=w0_sb[:], start=True, stop=True)
                    nc.vector.tensor_copy(out=st_sb[:full[i], i, 0:D],
                                          in_=ps[:full[i], 0:D])
                    nc.scalar.activation(
                        out=st_sb[:full[i], i, D:WB].bitcast(BF16),
                        in_=ps[:full[i], D:ROW],
                        func=mybir.ActivationFunctionType.Copy)
                if rows == gn * 128:
                    nc.sync.dma_start(
                        out=table1[r0:r0 + rows, :WB].rearrange("(i p) c -> p i c", p=128),
                        in_=st_sb[:, :gn, :WB])
                else:
                    for i in range(gn):
                        if full[i]:
                            nc.sync.dma_start(
                                out=table1[r0 + i * 128: r0 + i * 128 + full[i], :WB],
                                in_=st_sb[:full[i], i, :WB])
                t += gn

        ctx_pj.__exit__(None, None, None)
        # aldst slab for layer 1: own-shard rows of the replicated table1
        pid = nc.partition_id()
        own0 = pid * g.NPC
        nfull = 128 * (NBLK - 1)
        nc.vector.memset(ald1[:], 0)
        nc.scalar.dma_start(
            out=ald1[:, :NBLK - 1, :],
            in_=table1[bass.ds(own0, nfull), D + 2 * H:WB].bitcast(BF16)
                .rearrange("(b p) c -> p b c", p=128))
        nc.scalar.dma_start(
            out=ald1[:g.LASTB, NBLK - 1, :],
            in_=table1[bass.ds(own0 + nfull, g.LASTB), D + 2 * H:WB].bitcast(BF16))

        # ---------------- edge phase (both layers) ---------------------------
        def edge_phase(tbl, aldst_sb, layer, self_src):
            ch_off = 0
            o16_lo = 0
            o16_hi = 0
            qn = 0
            pool_ps = [None] * KC
            with tc.tile_pool(name=f"eg{layer}", bufs=4) as eg, \
                 tc.tile_pool(name=f"es{layer}", bufs=3) as es, \
                 tc.tile_pool(name=f"er{layer}", bufs=8) as er, \
                 tc.tile_pool(name=f"ep{layer}", bufs=2, space="PSUM") as epp, \
                 tc.tile_pool(name=f"ea{layer}", bufs=2, space="PSUM") as eap, \
                 tc.tile_pool(name=f"et{layer}", bufs=(2 if layer == 0 else 1), space="PSUM") as etp, \
                 tc.tile_pool(name=f"etr{layer}", bufs=2, space="PSUM") as etr:
                for b in range(NBLK):
                    Kl, Kh = K_lo[b], K_hi[b]
                    KT = 1 + Kl + Kh
                    S_all = es.tile([128, KMXT, 128], BF16, tag="sall")
                    nc.vector.tensor_tensor(
                        out=S_all[:, :KT, :],
                        in0=dcol_sb[:, ch_off:ch_off + KT, None].to_broadcast([128, KT, 128]),
                        in1=iota_rep[:, None, :].to_broadcast([128, KT, 128]),
                        op=mybir.AluOpType.is_equal)
                    ST_all = es.tile([128, KMXT, 128], BF16, tag="stall")
                    for j in range(KT):
                        stp = etr.tile([128, 128], BF16, tag="stp")
                        nc.tensor.transpose(out=stp[:], in_=S_all[:, j, :],
                                            identity=identb[:])
                        nc.scalar.activation(out=ST_all[:, j, :], in_=stp[:],
                                             func=mybir.ActivationFunctionType.Copy)
                    gl = eg.tile([128, 1 + KMAXL, RB], FP8, tag="glo")
                    gh = eg.tile([128, KMAXH, RB], FP8, tag="ghi")
                    if b < 4:  # NaN shield: zero never-written pad/trim lanes
                        nc.vector.memset(gl[:], 0)
                        nc.vector.memset(gh[:], 0)
                    # dense self-loop chunk: own-shard rows are contiguous
                    nc.sync.dma_start(out=gl[:sizes[b], 0, :WB], in_=self_src(b))
                    nc.gpsimd.dma_gather(
                        out_ap=gl[:, 1:1 + Kl, :], in_ap=tbl[0:g.LOROWS, :],
                        idxs_ap=il_all[:, o16_lo:o16_lo + Kl * 8], num_idxs=Kl * 128,
                        num_idxs_reg=Kl * 128, elem_size=RB, single_packet=False,
                        queue_num=qn % NQ)
                    nc.gpsimd.dma_gather(
                        out_ap=gh[:, :Kh, :], in_ap=tbl[g.LOROWS:g.N, :],
                        idxs_ap=ih_all[:, o16_hi:o16_hi + Kh * 8], num_idxs=Kh * 128,
                        num_idxs_reg=Kh * 128, elem_size=RB, single_packet=False,
                        queue_num=(qn + 1) % NQ)
                    qn += 2

                    alps = eap.tile([128, KMXT, H], F32, tag="alps")
                    bps = epp.tile([128, D + H], F32, tag="bps")  # [msgs | den]
                    lgall = er.tile([128, KMXT, H], F32, tag="lgall")
                    exal = er.tile([128, KMXT, H], F32, tag="exal")
                    for j in range(KT):
                        nc.tensor.matmul(out=alps[:, j, :], lhsT=ST_all[:, j, :],
                                         rhs=aldst_sb[:, b, :], start=True, stop=True)
                    nc.vector.tensor_tensor(
                        out=lgall[:, 0:1 + Kl, :],
                        in0=gl[:, 0:1 + Kl, D:WB].bitcast(BF16)[:, :, 0:H],
                        in1=alps[:, 0:1 + Kl, :], op=mybir.AluOpType.add)
                    nc.vector.tensor_tensor(
                        out=lgall[:, 1 + Kl:KT, :],
                        in0=gh[:, 0:Kh, D:WB].bitcast(BF16)[:, :, 0:H],
                        in1=alps[:, 1 + Kl:KT, :], op=mybir.AluOpType.add)
                    nc.vector.scalar_tensor_tensor(
                        out=lgall[:, :KT, :], in0=lgall[:, :KT, :],
                        scalar=NEG_SLOPE, in1=lgall[:, :KT, :],
                        op0=mybir.AluOpType.mult, op1=mybir.AluOpType.max)
                    nc.scalar.activation(out=exal[:, :KT, :], in_=lgall[:, :KT, :],
                                         func=mybir.ActivationFunctionType.Exp)
                    for j in range(KT):
                        gt, jj = (gl, j) if j <= Kl else (gh, j - 1 - Kl)
                        rhs = er.tile([128, D + H], BF16, tag="rhs")
                        nc.scalar.activation(out=rhs[:, D:D + H], in_=exal[:, j, :],
                                             func=mybir.ActivationFunctionType.Copy)
                        hh = 3
                        nc.vector.tensor_tensor(
                            out=rhs[:, 0:hh * g.C].rearrange("p (h c) -> p h c", h=hh),
                            in0=gt[:, jj, 0:hh * g.C].rearrange("p (h c) -> p h c", h=hh),
                            in1=rhs[:, D:D + hh][:, :, None].to_broadcast([128, hh, g.C]),
                            op=mybir.AluOpType.mult)
                        nc.scalar.activation(
                            out=rhs[:, hh * g.C:D],
                            in_=gt[:, jj, hh * g.C:D],
                            func=mybir.ActivationFunctionType.Copy,
                            scale=exal[:, j, hh:H])
                        nc.tensor.matmul(out=bps[:], lhsT=S_all[:, j, :], rhs=rhs[:],
                                         start=(j == 0), stop=(j == KT - 1))

                    rcp = er.tile([128, H], F32, tag="rcp")
                    nc.vector.reciprocal(rcp[:], bps[:, D:D + H])
                    hv = er.tile([128, D], F32, tag="hv")
                    nc.vector.tensor_tensor(
                        out=hv[:].rearrange("p (h c) -> p h c", h=H),
                        in0=bps[:, 0:D].rearrange("p (h c) -> p h c", h=H),
                        in1=rcp[:, :, None].to_broadcast([128, H, g.C]),
                        op=mybir.AluOpType.mult)
                    bias = b0_sb if layer == 0 else b1_sb
                    nc.vector.tensor_tensor(out=hv[:], in0=hv[:], in1=bias[:],
                                            op=mybir.AluOpType.add)
                    if layer == 0:
                        um = er.tile([128, D], F32, tag="um")
                        nc.vector.tensor_tensor(
                            out=um[:], in0=hv[:],
                            in1=zero_c[:].to_broadcast([128, D]),
                            op=mybir.AluOpType.min)
                        nc.scalar.activation(out=um[:], in_=um[:],
                                             func=mybir.ActivationFunctionType.Exp)
                        nc.vector.scalar_tensor_tensor(
                            out=hv[:], in0=hv[:], scalar=0.0, in1=um[:],
                            op0=mybir.AluOpType.max, op1=mybir.AluOpType.add)
                        nc.vector.tensor_tensor(
                            out=hv[:], in0=hv[:],
                            in1=ones_f[:].to_broadcast([128, D]),
                            op=mybir.AluOpType.subtract)
                        for kc in range(KC):
                            tps = etp.tile([128, 128], F32, tag="tps")
                            nc.tensor.transpose(
                                out=tps[:], in_=hv[:, kc * 128:(kc + 1) * 128],
                                identity=ident[:])
                            nc.vector.tensor_copy(out=stash[:, kc, b, :], in_=tps[:])
                    else:
                        ov = ones_f if sizes[b] == 128 else ones_p
                        for kc in range(KC):
                            if pool_ps[kc] is None:
                                pps = etp.tile([128, 1], F32, tag=f"pool{kc}")
                                pool_ps[kc] = pps
                            nc.tensor.matmul(
                                out=pool_ps[kc][:],
                                lhsT=hv[:, kc * 128:(kc + 1) * 128],
                                rhs=ov[:], start=(b == 0), stop=(b == NBLK - 1))
                    ch_off += KT
                    o16_lo += Kl * 8
                    o16_hi += Kh * 8
                if layer == 1:
                    for kc in range(KC):
                        nc.vector.tensor_copy(out=pool_sb[:, kc:kc + 1],
                                              in_=pool_ps[kc][:])

        with nc.named_scope("l1edge"):
            edge_phase(table1, ald1, 0,
                       lambda b: table1[bass.ds(own0 + b * 128, sizes[b]), :WB])

        # ---------------- phase 3: L2 projection (own shard) -----------------
        ctx_l2 = nc.named_scope("l2proj"); ctx_l2.__enter__()
        with tc.tile_pool(name="l2", bufs=3) as l2, \
             tc.tile_pool(name="l2p", bufs=2, space="PSUM") as l2p:
            for b in range(NBLK):
                ps = l2p.tile([128, ROW], F32, tag="pp2")
                for kc in range(KC):
                    nc.tensor.matmul(out=ps[:], lhsT=stash[:, kc, b, :],
                                     rhs=w1_sb[:, kc, :],
                                     start=(kc == 0), stop=(kc == KC - 1))
                sb = l2.tile([128, RB], FP8, tag="sb2")
                nc.vector.tensor_copy(out=sb[:sizes[b], 0:D], in_=ps[:sizes[b], 0:D])
                nc.scalar.activation(
                    out=sb[:sizes[b], D:WB].bitcast(BF16),
                    in_=ps[:sizes[b], D:ROW],
                    func=mybir.ActivationFunctionType.Copy)
                nc.sync.dma_start(out=l2loc[b * 128: b * 128 + sizes[b], :WB],
                                  in_=sb[:sizes[b], :WB])

        ctx_l2.__exit__(None, None, None)
        nc.vector.memset(ald2[:], 0)
        nc.scalar.dma_start(
            out=ald2[:, :NBLK - 1, :],
            in_=l2loc[0:nfull, D + 2 * H:WB].bitcast(BF16)
                .rearrange("(b p) c -> p b c", p=128))
        nc.scalar.dma_start(
            out=ald2[:g.LASTB, NBLK - 1, :],
            in_=l2loc[nfull:g.NPC, D + 2 * H:WB].bitcast(BF16))

        # ---------------- phase 4: AllGather L2 table ------------------------
        with nc.named_scope("allgather"):
            nc.gpsimd.collective_compute(
                "AllGather", mybir.AluOpType.bypass,
                replica_groups=[list(range(g.ncores))],
                ins=[l2loc[:]], outs=[table2[:]])

        # ---------------- phase 5: L2 edge + pool ----------------------------
        with nc.named_scope("l2edge"):
            edge_phase(table2, ald2, 1,
                       lambda b: l2loc[b * 128: b * 128 + sizes[b], :WB])

        # ---------------- phase 6: AllReduce + classifier --------------------
        nc.sync.dma_start(out=ar_in[:], in_=pool_sb[:])
        nc.gpsimd.collective_compute(
            "AllReduce", mybir.AluOpType.add,
            replica_groups=[list(range(g.ncores))],
            ins=[ar_in[:]], outs=[ar_out[:]])
        with tc.tile_pool(name="fin", bufs=1) as fin, \
             tc.tile_pool(name="finp", bufs=1, space="PSUM") as finp:
            pooled = fin.tile([128, KC], F32)
            nc.sync.dma_start(out=pooled[:], in_=ar_out[:])
            lgp = finp.tile([1, g.NCLS], F32, tag="lgp")
            for kc in range(KC):
                nc.tensor.matmul(out=lgp[:], lhsT=pooled[:, kc:kc + 1],
                                 rhs=clsw_sb[:, kc, :],
                                 start=(kc == 0), stop=(kc == KC - 1))
            lgs = fin.tile([1, g.NCLS], F32)
            nc.vector.tensor_scalar_mul(lgs[:], lgp[:], 1.0 / g.N)
            nc.vector.tensor_tensor(out=lgs[:], in0=lgs[:], in1=clsb_sb[:],
                                    op=mybir.AluOpType.add)
            nc.scalar.activation(out=lgs[:], in_=lgs[:],
                                 func=mybir.ActivationFunctionType.Exp)
            ssum = fin.tile([1, 1], F32)
            nc.vector.tensor_reduce(out=ssum[:], in_=lgs[:],
                                    axis=mybir.AxisListType.X, op=mybir.AluOpType.add)
            rs = fin.tile([1, 1], F32)
            nc.vector.reciprocal(rs[:], ssum[:])
            nc.vector.tensor_tensor(out=lgs[:], in0=lgs[:],
                                    in1=rs[:].to_broadcast([1, g.NCLS]),
                                    op=mybir.AluOpType.mult)
            nc.sync.dma_start(out=out_ext[:], in_=lgs[:])
    nc.finalize()
    return nc


# ----------------------------------------------------------------------------
# host entry
# ----------------------------------------------------------------------------

def _fold_weights(W, a_src, a_dst):
    H, C = a_src.shape
    D = W.shape[1]
    Asrc = np.zeros((D, H), np.float32)
    Adst = np.zeros((D, H), np.float32)
    for h in range(H):
        Asrc[h * C:(h + 1) * C, h] = a_src[h]
        Adst[h * C:(h + 1) * C, h] = a_dst[h]
    return np.concatenate([W, W @ Asrc, W @ Adst], axis=1)  # [in, D+2H]


def kernel(x, edge_index, W0, a0_src, a0_dst, b0, W1, a1_src, a1_dst, b1,
           cls_W, cls_b):
    x = np.asarray(x, np.float32)
    edge_index = np.asarray(edge_index, np.int64)
    N, F = x.shape
    a0_src = np.asarray(a0_src, np.float32)
    H, C = a0_src.shape
    cls_b = np.asarray(cls_b, np.float32)
    NCLS = cls_b.shape[0]
    g = Geo(N, F, H, C, NCLS)
    KC = g.D // 128

    pi, inv_pi, K_lo, K_hi, metas = preprocess(edge_index, g)

    bf = lambda a: np.ascontiguousarray(np.asarray(a, np.float32).astype(ml_dtypes.bfloat16))
    W0p = bf(_fold_weights(np.asarray(W0, np.float32), a0_src,
                           np.asarray(a0_dst, np.float32)))
    W1f = _fold_weights(np.asarray(W1, np.float32), np.asarray(a1_src, np.float32),
                        np.asarray(a1_dst, np.float32))
    W1p = bf(W1f.reshape(KC, 128, g.ROW).transpose(1, 0, 2))
    xTb = bf(x[inv_pi].T)

    consts = np.zeros((128, 260), np.float32)
    consts[:, 0] = np.arange(128)
    consts[:, 1:129] = np.eye(128)
    consts[:, 129] = 1.0
    consts[:g.LASTB, 130] = 1.0
    consts[:, 131:259] = np.arange(128)[None, :]
    clsWr = np.ascontiguousarray(
        np.asarray(cls_W, np.float32).reshape(KC, 128, NCLS).transpose(1, 0, 2))

    common = dict(
        xT=xTb, W0p=W0p, W1p=W1p,
        b0r=np.tile(np.asarray(b0, np.float32)[None, :], (128, 1)),
        b1r=np.tile(np.asarray(b1, np.float32)[None, :], (128, 1)),
        clsW=clsWr, clsb=cls_b[None, :],
        consts=consts,
    )
    in_maps = [dict(common, **metas[k]) for k in range(g.ncores)]

    nc = build_program(g, K_lo, K_hi)
    res = run_bass_kernel_spmd(nc, in_maps, list(range(g.ncores)))
    return np.asarray(res.results[0]["out"], np.float32)
